# revision 4
# baseline (speedup 1.0000x reference)
"""ChildSum TreeGRU on 8 Trainium2 NeuronCores.

Data-parallel over trees (16 trees/core). On-device layout is feature-major
([256 feat] -> 2x128 partitions, nodes on the free dim); the host transposes
x's leaf slice in and the output back out. All matmuls run as float32r.

Heap tree, per-core column order is tree-major: col = tree*len + in-level pos.
Levels 10(leaves)..6 are processed per group of 4 trees; level-6 results land
in a joint buffer [128, 16*127] covering heap nodes 0..126 (levels 0..6) for
all 16 trees; levels 5..0 are then processed jointly and the buffer is DMA'd
out in one shot.
"""
import sys

for p in ("/opt/trn_rl_repo", "/root/.axon_site/_ro/trn_rl_repo"):
    if p not in sys.path:
        sys.path.insert(0, p)

import numpy as np
import concourse.tile as tile
from concourse import bacc, mybir
from concourse.bass_utils import run_bass_kernel_spmd

f32 = mybir.dt.float32
f32r = mybir.dt.float32r
AF = mybir.ActivationFunctionType
ALU = mybir.AluOpType

T, DEPTH, NN, H = 128, 11, 2047, 256
NCORES = 8
TPC = T // NCORES          # 16 trees per core
G = 4                      # trees per group
NG = TPC // G              # 4 groups
NLEAF = 1 << (DEPTH - 1)   # 1024
LEAF0 = NLEAF - 1          # 1023
JN = (1 << 7) - 1          # 127 nodes/tree in the joint buffer (levels 0..6)
PS_COLS = 2048             # psum batch (4 banks) consumed by one ACT


def _emit_level(nc, P, tag, NT, Lct, hc3, hc_flat, out3, Wt, bias):
    """One GRU level for NT trees with Lct children per tree.

    hc3:    child-state AP [128, NT, Lct] per half (f32r)
    hc_flat: contiguous 2D view [128, NT*Lct] per half, or None (jbuf)
    out3:   output AP [128, NT, Lpt] per half (f32r)
    """
    Lc = NT * Lct
    Lp = Lc // 2
    uzT, urT, ucT = Wt["uz"], Wt["ur"], Wt["uc"]
    bz, br, bc = bias["bz"], bias["br"], bias["bc"]

    def mm_into(ps, off, lhs, rhs_pair):
        # accumulate both K-halves of one <=512-col chunk into ps[:, off:...]
        n = rhs_pair[0].free_size()
        nc.tensor.matmul(ps[:, off:off + n], lhs[0], rhs_pair[0], start=True, stop=False)
        nc.tensor.matmul(ps[:, off:off + n], lhs[1], rhs_pair[1], start=False, stop=True)

    def child_chunks():
        # yield (cols_off, [rhs_half0, rhs_half1]) chunks of <=512 child cols
        if hc_flat is not None:
            for c0 in range(0, Lc, 512):
                n = min(512, Lc - c0)
                yield c0, [hc_flat[k][:, c0:c0 + n] for k in range(2)]
        else:
            tch = max(1, 512 // Lct)
            for t0 in range(0, NT, tch):
                t1 = min(NT, t0 + tch)
                yield t0 * Lct, [hc3[k][:, t0:t1, :] for k in range(2)]

    # --- z = sigmoid(Uz @ hc + bz) over all children ---
    z = [P["z"].tile([128, Lc], f32, name=f"z{tag}_{m}", tag=f"z{m}") for m in range(2)]
    for m in range(2):
        lhs = [uzT[k][:, m * 128:(m + 1) * 128] for k in range(2)]
        for p0 in range(0, Lc, PS_COLS):
            pn = min(PS_COLS, Lc - p0)
            ps = P["ps"].tile([128, pn], f32, name=f"psz{tag}_{m}_{p0}", tag="ps")
            for c0, rhs in child_chunks():
                if p0 <= c0 < p0 + pn:
                    mm_into(ps, c0 - p0, lhs, rhs)
            nc.scalar.activation(z[m][:, p0:p0 + pn], ps[:], AF.Sigmoid, bias=bz[m][:])

    # --- h_sum = hc_even + hc_odd (strided), f32r ---
    hs = [P["hs"].tile([128, Lp], f32r, name=f"hs{tag}_{m}", tag=f"hs{m}") for m in range(2)]
    for m in range(2):
        nc.vector.tensor_tensor(hs[m][:], hc3[m][:, :, 0::2], hc3[m][:, :, 1::2], ALU.add)

    # --- r = sigmoid(Ur @ h_sum + br) ---
    r = [P["r"].tile([128, Lp], f32, name=f"r{tag}_{m}", tag=f"r{m}") for m in range(2)]
    for m in range(2):
        lhs = [urT[k][:, m * 128:(m + 1) * 128] for k in range(2)]
        for p0 in range(0, Lp, PS_COLS):
            pn = min(PS_COLS, Lp - p0)
            ps = P["ps"].tile([128, pn], f32, name=f"psr{tag}_{m}_{p0}", tag="ps")
            for c0 in range(p0, p0 + pn, 512):
                n = min(512, p0 + pn - c0)
                mm_into(ps, c0 - p0, lhs, [hs[k][:, c0:c0 + n] for k in range(2)])
            nc.scalar.activation(r[m][:, p0:p0 + pn], ps[:], AF.Sigmoid, bias=br[m][:])

    # --- rh = r * h_sum (in place into hs, stays f32r) ---
    for m in range(2):
        nc.vector.tensor_tensor(hs[m][:], r[m][:], hs[m][:], ALU.mult)

    # --- h_cand = tanh(Uc @ rh + bc) ---
    hcand = [P["hc"].tile([128, Lp], f32, name=f"hcand{tag}_{m}", tag=f"hcand{m}") for m in range(2)]
    for m in range(2):
        lhs = [ucT[k][:, m * 128:(m + 1) * 128] for k in range(2)]
        for p0 in range(0, Lp, PS_COLS):
            pn = min(PS_COLS, Lp - p0)
            ps = P["ps"].tile([128, pn], f32, name=f"psc{tag}_{m}_{p0}", tag="ps")
            for c0 in range(p0, p0 + pn, 512):
                n = min(512, p0 + pn - c0)
                mm_into(ps, c0 - p0, lhs, [hs[k][:, c0:c0 + n] for k in range(2)])
            nc.scalar.activation(hcand[m][:, p0:p0 + pn], ps[:], AF.Tanh, bias=bc[m][:])

    for m in range(2):
        z3 = z[m][:].rearrange("p (t n) -> p t n", t=NT)
        # zs = z_even + z_odd  (before z is overwritten by zh); reuses the r slot
        zs = P["r"].tile([128, Lp], f32, name=f"zs{tag}_{m}", tag=f"r{m}")
        nc.vector.tensor_tensor(zs[:], z3[:, :, 0::2], z3[:, :, 1::2], ALU.add)
        # zh = z * hc, in place into z (gpsimd offload)
        nc.gpsimd.tensor_tensor(z[m][:], z[m][:], hc3[m].bitcast(f32), ALU.mult)
        # zh_sum = zh_even + zh_odd; reuses the h_sum slot
        zhs = P["hs"].tile([128, Lp], f32, name=f"zhs{tag}_{m}", tag=f"hs{m}")
        nc.vector.tensor_tensor(zhs[:], z3[:, :, 0::2], z3[:, :, 1::2], ALU.add)
        # t = (zs - 1) * h_cand, in place into hcand
        nc.vector.scalar_tensor_tensor(hcand[m][:], zs[:], 1.0, hcand[m][:], ALU.subtract, ALU.mult)
        # h_new = zh_sum - t  -> out3 (f32r)
        nc.vector.tensor_tensor(out3[m], zhs[:], hcand[m][:], ALU.subtract)


def _build():
    nc = bacc.Bacc("TRN2", debug=False)

    xT_d = nc.dram_tensor("xT", [H, TPC * NLEAF], f32r, kind="ExternalInput")
    wT_d = nc.dram_tensor("wT", [H, H], f32r, kind="ExternalInput")
    uzT_d = nc.dram_tensor("uzT", [H, H], f32r, kind="ExternalInput")
    urT_d = nc.dram_tensor("urT", [H, H], f32r, kind="ExternalInput")
    ucT_d = nc.dram_tensor("ucT", [H, H], f32r, kind="ExternalInput")
    bw_d = nc.dram_tensor("bw", [H, 1], f32, kind="ExternalInput")
    bz_d = nc.dram_tensor("bz", [H, 1], f32, kind="ExternalInput")
    br_d = nc.dram_tensor("br", [H, 1], f32, kind="ExternalInput")
    bc_d = nc.dram_tensor("bc", [H, 1], f32, kind="ExternalInput")
    hout_d = nc.dram_tensor("h_out", [H, TPC, NN], f32, kind="ExternalOutput")

    with tile.TileContext(nc) as tc:
        from contextlib import ExitStack
        with ExitStack() as ctx:
            P = {}
            P["const"] = ctx.enter_context(tc.tile_pool(name="const", bufs=1))
            P["xg"] = ctx.enter_context(tc.tile_pool(name="xg", bufs=2))
            P["h10"] = ctx.enter_context(tc.tile_pool(name="h10", bufs=1))
            P["hl"] = ctx.enter_context(tc.tile_pool(name="hl", bufs=1))
            P["jbuf"] = ctx.enter_context(tc.tile_pool(name="jbuf", bufs=1))
            P["z"] = ctx.enter_context(tc.tile_pool(name="z", bufs=1))
            P["hs"] = ctx.enter_context(tc.tile_pool(name="hs", bufs=1))
            P["r"] = ctx.enter_context(tc.tile_pool(name="r", bufs=1))
            P["hc"] = ctx.enter_context(tc.tile_pool(name="hc", bufs=1))
            P["ps"] = ctx.enter_context(tc.tile_pool(name="ps", bufs=2, space="PSUM"))

            cp = P["const"]
            Wt = {}
            for nm, d in (("w", wT_d), ("uz", uzT_d), ("ur", urT_d), ("uc", ucT_d)):
                Wt[nm] = [cp.tile([128, H], f32r, name=f"{nm}T{k}") for k in range(2)]
                for k in range(2):
                    nc.sync.dma_start(Wt[nm][k][:], d.ap()[k * 128:(k + 1) * 128, :])
            bias = {}
            for nm, d in (("bw", bw_d), ("bz", bz_d), ("br", br_d), ("bc", bc_d)):
                bias[nm] = [cp.tile([128, 1], f32, name=f"{nm}{m}") for m in range(2)]
                for m in range(2):
                    nc.sync.dma_start(bias[nm][m][:], d.ap()[m * 128:(m + 1) * 128, :])

            # joint buffer: heap nodes 0..126 for all 16 trees, per half
            jbuf = [P["jbuf"].tile([128, TPC * JN], f32r, name=f"jbuf{m}") for m in range(2)]
            jv = [jbuf[m][:].rearrange("p (t n) -> p t n", t=TPC) for m in range(2)]

            for g in range(NG):
                gt = f"g{g}"
                # ---- leaf phase: h10 = tanh(W @ x + bw) ----
                xg = [P["xg"].tile([128, G * NLEAF], f32r, name=f"x{gt}_{k}", tag="xg")
                      for k in range(2)]
                for k in range(2):
                    nc.sync.dma_start(
                        xg[k][:],
                        xT_d.ap()[k * 128:(k + 1) * 128,
                                  g * G * NLEAF:(g + 1) * G * NLEAF])
                h10 = [P["h10"].tile([128, G * NLEAF], f32r, name=f"h10{gt}_{m}", tag=f"h10{m}")
                       for m in range(2)]
                for m in range(2):
                    lhs = [Wt["w"][k][:, m * 128:(m + 1) * 128] for k in range(2)]
                    for p0 in range(0, G * NLEAF, PS_COLS):
                        pn = min(PS_COLS, G * NLEAF - p0)
                        ps = P["ps"].tile([128, pn], f32, name=f"psx{gt}_{m}_{p0}", tag="ps")
                        for c0 in range(p0, p0 + pn, 512):
                            n = min(512, p0 + pn - c0)
                            nc.tensor.matmul(ps[:, c0 - p0:c0 - p0 + n], lhs[0],
                                             xg[0][:, c0:c0 + n], start=True, stop=False)
                            nc.tensor.matmul(ps[:, c0 - p0:c0 - p0 + n], lhs[1],
                                             xg[1][:, c0:c0 + n], start=False, stop=True)
                        nc.scalar.activation(h10[m][:, p0:p0 + pn], ps[:], AF.Tanh,
                                             bias=bias["bw"][m][:])
                    nc.sync.dma_start(
                        hout_d.ap()[m * 128:(m + 1) * 128, g * G:(g + 1) * G,
                                    LEAF0:LEAF0 + NLEAF],
                        h10[m][:].rearrange("p (t n) -> p t n", t=G).bitcast(f32))

                # ---- levels 9..6 for this group ----
                hchild = h10
                for lv in range(DEPTH - 2, 5, -1):
                    Lct = 2 ** (lv + 1)
                    Lpt = 2 ** lv
                    hc3 = [hchild[m][:].rearrange("p (t n) -> p t n", t=G) for m in range(2)]
                    hc_flat = [hchild[m][:] for m in range(2)]
                    if lv == 6:
                        out3 = [jv[m][:, g * G:(g + 1) * G, Lpt - 1:2 * Lpt - 1]
                                for m in range(2)]
                    else:
                        hnew = [P["hl"].tile([128, G * Lpt], f32r,
                                             name=f"h{lv}{gt}_{m}", tag=f"h{lv}_{m}")
                                for m in range(2)]
                        out3 = [hnew[m][:].rearrange("p (t n) -> p t n", t=G)
                                for m in range(2)]
                    _emit_level(nc, P, f"{gt}l{lv}", G, Lct, hc3, hc_flat, out3, Wt, bias)
                    if lv > 6:
                        for m in range(2):
                            nc.sync.dma_start(
                                hout_d.ap()[m * 128:(m + 1) * 128, g * G:(g + 1) * G,
                                            Lpt - 1:2 * Lpt - 1],
                                hnew[m][:].rearrange("p (t n) -> p t n", t=G).bitcast(f32))
                        hchild = hnew

            # ---- joint levels 5..0 over jbuf ----
            for lv in range(5, -1, -1):
                Lct = 2 ** (lv + 1)
                Lpt = 2 ** lv
                hc3 = [jv[m][:, :, Lct - 1:2 * Lct - 1] for m in range(2)]
                out3 = [jv[m][:, :, Lpt - 1:2 * Lpt - 1] for m in range(2)]
                _emit_level(nc, P, f"j{lv}", TPC, Lct, hc3, None, out3, Wt, bias)

            for m in range(2):
                nc.sync.dma_start(
                    hout_d.ap()[m * 128:(m + 1) * 128, :, 0:JN],
                    jv[m].bitcast(f32))

    nc.compile()
    return nc


_NC = None


def _get_nc():
    global _NC
    if _NC is None:
        _NC = _build()
    return _NC


def make_in_maps(inputs):
    x = np.asarray(inputs["x"], np.float32)
    W = np.asarray(inputs["W"], np.float32)
    bW = np.asarray(inputs["bW"], np.float32).reshape(H, 1)
    Ur = np.asarray(inputs["Ur"], np.float32)
    br = np.asarray(inputs["br"], np.float32).reshape(H, 1)
    Uc = np.asarray(inputs["Uc"], np.float32)
    bc = np.asarray(inputs["bc"], np.float32).reshape(H, 1)
    Uz = np.asarray(inputs["Uz"], np.float32)
    bz = np.asarray(inputs["bz"], np.float32).reshape(H, 1)
    shared = {
        "wT": np.ascontiguousarray(W.T), "uzT": np.ascontiguousarray(Uz.T),
        "urT": np.ascontiguousarray(Ur.T), "ucT": np.ascontiguousarray(Uc.T),
        "bw": bW, "bz": bz, "br": br, "bc": bc,
    }
    in_maps = []
    for c in range(NCORES):
        xs = x[c * TPC:(c + 1) * TPC, LEAF0:, :]          # [16, 1024, 256]
        xTc = np.ascontiguousarray(xs.transpose(2, 0, 1)).reshape(H, TPC * NLEAF)
        in_maps.append({"xT": xTc, **shared})
    return in_maps


def assemble_out(core_outs):
    out = np.empty((T, NN, H), np.float32)
    for c in range(NCORES):
        # [256, 16, 2047] -> [16, 2047, 256]
        out[c * TPC:(c + 1) * TPC] = core_outs[c].transpose(1, 2, 0)
    return out


def kernel(**inputs):
    nc = _get_nc()
    in_maps = make_in_maps(inputs)
    res = run_bass_kernel_spmd(nc, in_maps, list(range(NCORES)))
    return assemble_out([r["h_out"] for r in res.results])


# revision 5
# speedup vs baseline: 1.1213x; 1.1213x over previous
"""ChildSum TreeGRU on 8 Trainium2 NeuronCores.

Data-parallel over trees (16 trees/core). On-device layout is feature-major
([256 feat] -> 2x128 partitions, nodes on the free dim); the host transposes
x's leaf slice in and the output back out. All matmuls run as float32r.

Heap tree, per-core column order is tree-major: col = tree*len + in-level pos.
Levels 10(leaves)..6 are processed per group of 4 trees; level-6 results land
in a joint buffer [128, 16*127] covering heap nodes 0..126 (levels 0..6) for
all 16 trees; levels 5..0 are then processed jointly and the buffer is DMA'd
out in one shot.
"""
import sys

for p in ("/opt/trn_rl_repo", "/root/.axon_site/_ro/trn_rl_repo"):
    if p not in sys.path:
        sys.path.insert(0, p)

import numpy as np
import concourse.tile as tile
from concourse import bacc, mybir
from concourse.bass_utils import run_bass_kernel_spmd

f32 = mybir.dt.float32
f32r = mybir.dt.float32r
AF = mybir.ActivationFunctionType
ALU = mybir.AluOpType

T, DEPTH, NN, H = 128, 11, 2047, 256
NCORES = 8
TPC = T // NCORES          # 16 trees per core
G = 4                      # trees per group
NG = TPC // G              # 4 groups
NLEAF = 1 << (DEPTH - 1)   # 1024
LEAF0 = NLEAF - 1          # 1023
JN = (1 << 7) - 1          # 127 nodes/tree in the joint buffer (levels 0..6)
PS_COLS = 2048             # psum batch (4 banks) consumed by one ACT


def _emit_level(nc, P, tag, NT, Lct, hc3, hc_flat, out3, Wt, bias):
    """One GRU level for NT trees with Lct children per tree.

    hc3:    child-state AP [128, NT, Lct] per half (f32r)
    hc_flat: contiguous 2D view [128, NT*Lct] per half, or None (jbuf)
    out3:   output AP [128, NT, Lpt] per half (f32r)
    """
    Lc = NT * Lct
    Lp = Lc // 2
    uzT, urT, ucT = Wt["uz"], Wt["ur"], Wt["uc"]
    bz, br, bc = bias["bz"], bias["br"], bias["bc"]

    def mm_into(ps, off, lhs, rhs_pair):
        # accumulate both K-halves of one <=512-col chunk into ps[:, off:...]
        n = rhs_pair[0].free_size()
        nc.tensor.matmul(ps[:, off:off + n], lhs[0], rhs_pair[0], start=True, stop=False)
        nc.tensor.matmul(ps[:, off:off + n], lhs[1], rhs_pair[1], start=False, stop=True)

    def child_chunks():
        # yield (cols_off, [rhs_half0, rhs_half1]) chunks of <=512 child cols
        if hc_flat is not None:
            for c0 in range(0, Lc, 512):
                n = min(512, Lc - c0)
                yield c0, [hc_flat[k][:, c0:c0 + n] for k in range(2)]
        else:
            tch = max(1, 512 // Lct)
            for t0 in range(0, NT, tch):
                t1 = min(NT, t0 + tch)
                yield t0 * Lct, [hc3[k][:, t0:t1, :] for k in range(2)]

    # --- z = sigmoid(Uz @ hc + bz) over all children ---
    z = [P["z"].tile([128, Lc], f32, name=f"z{tag}_{m}", tag=f"z{m}") for m in range(2)]
    for m in range(2):
        lhs = [uzT[k][:, m * 128:(m + 1) * 128] for k in range(2)]
        for p0 in range(0, Lc, PS_COLS):
            pn = min(PS_COLS, Lc - p0)
            ps = P["ps"].tile([128, pn], f32, name=f"psz{tag}_{m}_{p0}", tag="ps")
            for c0, rhs in child_chunks():
                if p0 <= c0 < p0 + pn:
                    mm_into(ps, c0 - p0, lhs, rhs)
            nc.scalar.activation(z[m][:, p0:p0 + pn], ps[:], AF.Sigmoid, bias=bz[m][:])

    # --- h_sum = hc_even + hc_odd (strided), f32r ---
    hs = [P["hs"].tile([128, Lp], f32r, name=f"hs{tag}_{m}", tag=f"hs{m}") for m in range(2)]
    for m in range(2):
        nc.vector.tensor_tensor(hs[m][:], hc3[m][:, :, 0::2], hc3[m][:, :, 1::2], ALU.add)

    # --- r = sigmoid(Ur @ h_sum + br) ---
    r = [P["r"].tile([128, Lp], f32, name=f"r{tag}_{m}", tag=f"r{m}") for m in range(2)]
    for m in range(2):
        lhs = [urT[k][:, m * 128:(m + 1) * 128] for k in range(2)]
        for p0 in range(0, Lp, PS_COLS):
            pn = min(PS_COLS, Lp - p0)
            ps = P["ps"].tile([128, pn], f32, name=f"psr{tag}_{m}_{p0}", tag="ps")
            for c0 in range(p0, p0 + pn, 512):
                n = min(512, p0 + pn - c0)
                mm_into(ps, c0 - p0, lhs, [hs[k][:, c0:c0 + n] for k in range(2)])
            nc.scalar.activation(r[m][:, p0:p0 + pn], ps[:], AF.Sigmoid, bias=br[m][:])

    # --- rh = r * h_sum (in place into hs, stays f32r) ---
    for m in range(2):
        nc.vector.tensor_tensor(hs[m][:], r[m][:], hs[m][:], ALU.mult)

    # --- h_cand = tanh(Uc @ rh + bc) ---
    hcand = [P["hc"].tile([128, Lp], f32, name=f"hcand{tag}_{m}", tag=f"hcand{m}") for m in range(2)]
    for m in range(2):
        lhs = [ucT[k][:, m * 128:(m + 1) * 128] for k in range(2)]
        for p0 in range(0, Lp, PS_COLS):
            pn = min(PS_COLS, Lp - p0)
            ps = P["ps"].tile([128, pn], f32, name=f"psc{tag}_{m}_{p0}", tag="ps")
            for c0 in range(p0, p0 + pn, 512):
                n = min(512, p0 + pn - c0)
                mm_into(ps, c0 - p0, lhs, [hs[k][:, c0:c0 + n] for k in range(2)])
            nc.scalar.activation(hcand[m][:, p0:p0 + pn], ps[:], AF.Tanh, bias=bc[m][:])

    for m in range(2):
        z3 = z[m][:].rearrange("p (t n) -> p t n", t=NT)
        # zs = z_even + z_odd  (before z is overwritten by zh); reuses the r slot
        zs = P["r"].tile([128, Lp], f32, name=f"zs{tag}_{m}", tag=f"r{m}")
        nc.vector.tensor_tensor(zs[:], z3[:, :, 0::2], z3[:, :, 1::2], ALU.add)
        # zh = z * hc, in place into z (DVE: gpsimd would contend for the
        # shared DVE/GpSimd SBUF port pair and slow both engines ~4x)
        nc.vector.tensor_tensor(z[m][:], z[m][:], hc3[m].bitcast(f32), ALU.mult)
        # zh_sum = zh_even + zh_odd; reuses the h_sum slot
        zhs = P["hs"].tile([128, Lp], f32, name=f"zhs{tag}_{m}", tag=f"hs{m}")
        nc.vector.tensor_tensor(zhs[:], z3[:, :, 0::2], z3[:, :, 1::2], ALU.add)
        # t = (zs - 1) * h_cand, in place into hcand
        nc.vector.scalar_tensor_tensor(hcand[m][:], zs[:], 1.0, hcand[m][:], ALU.subtract, ALU.mult)
        # h_new = zh_sum - t  -> out3 (f32r)
        nc.vector.tensor_tensor(out3[m], zhs[:], hcand[m][:], ALU.subtract)


def _build():
    nc = bacc.Bacc("TRN2", debug=False)

    xT_d = nc.dram_tensor("xT", [H, TPC * NLEAF], f32r, kind="ExternalInput")
    wT_d = nc.dram_tensor("wT", [H, H], f32r, kind="ExternalInput")
    uzT_d = nc.dram_tensor("uzT", [H, H], f32r, kind="ExternalInput")
    urT_d = nc.dram_tensor("urT", [H, H], f32r, kind="ExternalInput")
    ucT_d = nc.dram_tensor("ucT", [H, H], f32r, kind="ExternalInput")
    bw_d = nc.dram_tensor("bw", [H, 1], f32, kind="ExternalInput")
    bz_d = nc.dram_tensor("bz", [H, 1], f32, kind="ExternalInput")
    br_d = nc.dram_tensor("br", [H, 1], f32, kind="ExternalInput")
    bc_d = nc.dram_tensor("bc", [H, 1], f32, kind="ExternalInput")
    hout_d = nc.dram_tensor("h_out", [H, TPC, NN], f32, kind="ExternalOutput")

    with tile.TileContext(nc) as tc:
        from contextlib import ExitStack
        with ExitStack() as ctx:
            P = {}
            P["const"] = ctx.enter_context(tc.tile_pool(name="const", bufs=1))
            P["xg"] = ctx.enter_context(tc.tile_pool(name="xg", bufs=2))
            P["h10"] = ctx.enter_context(tc.tile_pool(name="h10", bufs=1))
            P["hl"] = ctx.enter_context(tc.tile_pool(name="hl", bufs=1))
            P["jbuf"] = ctx.enter_context(tc.tile_pool(name="jbuf", bufs=1))
            P["z"] = ctx.enter_context(tc.tile_pool(name="z", bufs=1))
            P["hs"] = ctx.enter_context(tc.tile_pool(name="hs", bufs=1))
            P["r"] = ctx.enter_context(tc.tile_pool(name="r", bufs=1))
            P["hc"] = ctx.enter_context(tc.tile_pool(name="hc", bufs=1))
            P["ps"] = ctx.enter_context(tc.tile_pool(name="ps", bufs=2, space="PSUM"))

            cp = P["const"]
            Wt = {}
            for nm, d in (("w", wT_d), ("uz", uzT_d), ("ur", urT_d), ("uc", ucT_d)):
                Wt[nm] = [cp.tile([128, H], f32r, name=f"{nm}T{k}") for k in range(2)]
                for k in range(2):
                    nc.sync.dma_start(Wt[nm][k][:], d.ap()[k * 128:(k + 1) * 128, :])
            bias = {}
            for nm, d in (("bw", bw_d), ("bz", bz_d), ("br", br_d), ("bc", bc_d)):
                bias[nm] = [cp.tile([128, 1], f32, name=f"{nm}{m}") for m in range(2)]
                for m in range(2):
                    nc.sync.dma_start(bias[nm][m][:], d.ap()[m * 128:(m + 1) * 128, :])

            # joint buffer: heap nodes 0..126 for all 16 trees, per half
            jbuf = [P["jbuf"].tile([128, TPC * JN], f32r, name=f"jbuf{m}") for m in range(2)]
            jv = [jbuf[m][:].rearrange("p (t n) -> p t n", t=TPC) for m in range(2)]

            for g in range(NG):
                gt = f"g{g}"
                # ---- leaf phase: h10 = tanh(W @ x + bw) ----
                xg = [P["xg"].tile([128, G * NLEAF], f32r, name=f"x{gt}_{k}", tag="xg")
                      for k in range(2)]
                for k in range(2):
                    nc.sync.dma_start(
                        xg[k][:],
                        xT_d.ap()[k * 128:(k + 1) * 128,
                                  g * G * NLEAF:(g + 1) * G * NLEAF])
                h10 = [P["h10"].tile([128, G * NLEAF], f32r, name=f"h10{gt}_{m}", tag=f"h10{m}")
                       for m in range(2)]
                for m in range(2):
                    lhs = [Wt["w"][k][:, m * 128:(m + 1) * 128] for k in range(2)]
                    for p0 in range(0, G * NLEAF, PS_COLS):
                        pn = min(PS_COLS, G * NLEAF - p0)
                        ps = P["ps"].tile([128, pn], f32, name=f"psx{gt}_{m}_{p0}", tag="ps")
                        for c0 in range(p0, p0 + pn, 512):
                            n = min(512, p0 + pn - c0)
                            nc.tensor.matmul(ps[:, c0 - p0:c0 - p0 + n], lhs[0],
                                             xg[0][:, c0:c0 + n], start=True, stop=False)
                            nc.tensor.matmul(ps[:, c0 - p0:c0 - p0 + n], lhs[1],
                                             xg[1][:, c0:c0 + n], start=False, stop=True)
                        nc.scalar.activation(h10[m][:, p0:p0 + pn], ps[:], AF.Tanh,
                                             bias=bias["bw"][m][:])
                    nc.sync.dma_start(
                        hout_d.ap()[m * 128:(m + 1) * 128, g * G:(g + 1) * G,
                                    LEAF0:LEAF0 + NLEAF],
                        h10[m][:].rearrange("p (t n) -> p t n", t=G).bitcast(f32))

                # ---- levels 9..6 for this group ----
                hchild = h10
                for lv in range(DEPTH - 2, 5, -1):
                    Lct = 2 ** (lv + 1)
                    Lpt = 2 ** lv
                    hc3 = [hchild[m][:].rearrange("p (t n) -> p t n", t=G) for m in range(2)]
                    hc_flat = [hchild[m][:] for m in range(2)]
                    if lv == 6:
                        out3 = [jv[m][:, g * G:(g + 1) * G, Lpt - 1:2 * Lpt - 1]
                                for m in range(2)]
                    else:
                        hnew = [P["hl"].tile([128, G * Lpt], f32r,
                                             name=f"h{lv}{gt}_{m}", tag=f"h{lv}_{m}")
                                for m in range(2)]
                        out3 = [hnew[m][:].rearrange("p (t n) -> p t n", t=G)
                                for m in range(2)]
                    _emit_level(nc, P, f"{gt}l{lv}", G, Lct, hc3, hc_flat, out3, Wt, bias)
                    if lv > 6:
                        for m in range(2):
                            nc.sync.dma_start(
                                hout_d.ap()[m * 128:(m + 1) * 128, g * G:(g + 1) * G,
                                            Lpt - 1:2 * Lpt - 1],
                                hnew[m][:].rearrange("p (t n) -> p t n", t=G).bitcast(f32))
                        hchild = hnew

            # ---- joint levels 5..0 over jbuf ----
            for lv in range(5, -1, -1):
                Lct = 2 ** (lv + 1)
                Lpt = 2 ** lv
                hc3 = [jv[m][:, :, Lct - 1:2 * Lct - 1] for m in range(2)]
                out3 = [jv[m][:, :, Lpt - 1:2 * Lpt - 1] for m in range(2)]
                _emit_level(nc, P, f"j{lv}", TPC, Lct, hc3, None, out3, Wt, bias)

            for m in range(2):
                nc.sync.dma_start(
                    hout_d.ap()[m * 128:(m + 1) * 128, :, 0:JN],
                    jv[m].bitcast(f32))

    nc.compile()
    return nc


_NC = None


def _get_nc():
    global _NC
    if _NC is None:
        _NC = _build()
    return _NC


def make_in_maps(inputs):
    x = np.asarray(inputs["x"], np.float32)
    W = np.asarray(inputs["W"], np.float32)
    bW = np.asarray(inputs["bW"], np.float32).reshape(H, 1)
    Ur = np.asarray(inputs["Ur"], np.float32)
    br = np.asarray(inputs["br"], np.float32).reshape(H, 1)
    Uc = np.asarray(inputs["Uc"], np.float32)
    bc = np.asarray(inputs["bc"], np.float32).reshape(H, 1)
    Uz = np.asarray(inputs["Uz"], np.float32)
    bz = np.asarray(inputs["bz"], np.float32).reshape(H, 1)
    shared = {
        "wT": np.ascontiguousarray(W.T), "uzT": np.ascontiguousarray(Uz.T),
        "urT": np.ascontiguousarray(Ur.T), "ucT": np.ascontiguousarray(Uc.T),
        "bw": bW, "bz": bz, "br": br, "bc": bc,
    }
    in_maps = []
    for c in range(NCORES):
        xs = x[c * TPC:(c + 1) * TPC, LEAF0:, :]          # [16, 1024, 256]
        xTc = np.ascontiguousarray(xs.transpose(2, 0, 1)).reshape(H, TPC * NLEAF)
        in_maps.append({"xT": xTc, **shared})
    return in_maps


def assemble_out(core_outs):
    out = np.empty((T, NN, H), np.float32)
    for c in range(NCORES):
        # [256, 16, 2047] -> [16, 2047, 256]
        out[c * TPC:(c + 1) * TPC] = core_outs[c].transpose(1, 2, 0)
    return out


def kernel(**inputs):
    nc = _get_nc()
    in_maps = make_in_maps(inputs)
    res = run_bass_kernel_spmd(nc, in_maps, list(range(NCORES)))
    return assemble_out([r["h_out"] for r in res.results])


# revision 8
# speedup vs baseline: 1.2610x; 1.1246x over previous
"""ChildSum TreeGRU on 8 Trainium2 NeuronCores.

Data-parallel over trees (16 trees/core). On-device layout is feature-major
([256 feat] -> 2x128 partitions, nodes on the free dim); the host transposes
x's leaf slice in and the output back out. All matmuls run as float32r.

Heap tree, per-core column order is tree-major: col = tree*len + in-level pos.
Levels 10(leaves)..6 are processed per group of 4 trees; level-6 results land
in a joint buffer [128, 16*127] covering heap nodes 0..126 (levels 0..6) for
all 16 trees; levels 5..0 are then processed jointly and the buffer is DMA'd
out in one shot.
"""
import sys

for p in ("/opt/trn_rl_repo", "/root/.axon_site/_ro/trn_rl_repo"):
    if p not in sys.path:
        sys.path.insert(0, p)

import numpy as np
import concourse.tile as tile
from concourse import bacc, mybir
from concourse.bass_utils import run_bass_kernel_spmd

f32 = mybir.dt.float32
f32r = mybir.dt.float32r
AF = mybir.ActivationFunctionType
ALU = mybir.AluOpType

T, DEPTH, NN, H = 128, 11, 2047, 256
NCORES = 8
TPC = T // NCORES          # 16 trees per core
G = 4                      # trees per group
NG = TPC // G              # 4 groups
NLEAF = 1 << (DEPTH - 1)   # 1024
LEAF0 = NLEAF - 1          # 1023
JN = (1 << 7) - 1          # 127 nodes/tree in the joint buffer (levels 0..6)
PS_COLS = 1024             # psum batch (2 banks) consumed by one ACT


def _emit_level(nc, P, tag, NT, Lct, hc3, hc_flat, out3, Wt, bias):
    """One GRU level for NT trees with Lct children per tree.

    hc3:    child-state AP [128, NT, Lct] per half (f32r)
    hc_flat: contiguous 2D view [128, NT*Lct] per half, or None (jbuf)
    out3:   output AP [128, NT, Lpt] per half (f32r)
    """
    Lc = NT * Lct
    Lp = Lc // 2
    uzT, urT, ucT = Wt["uz"], Wt["ur"], Wt["uc"]
    bz, br, bc = bias["bz"], bias["br"], bias["bc"]

    def mm_into(ps, off, lhs, rhs_pair):
        # accumulate both K-halves of one <=512-col chunk into ps[:, off:...]
        n = rhs_pair[0].free_size()
        nc.tensor.matmul(ps[:, off:off + n], lhs[0], rhs_pair[0], start=True, stop=False)
        nc.tensor.matmul(ps[:, off:off + n], lhs[1], rhs_pair[1], start=False, stop=True)

    def child_chunks():
        # yield (cols_off, [rhs_half0, rhs_half1]) chunks of <=512 child cols
        if hc_flat is not None:
            for c0 in range(0, Lc, 512):
                n = min(512, Lc - c0)
                yield c0, [hc_flat[k][:, c0:c0 + n] for k in range(2)]
        else:
            tch = max(1, 512 // Lct)
            for t0 in range(0, NT, tch):
                t1 = min(NT, t0 + tch)
                yield t0 * Lct, [hc3[k][:, t0:t1, :] for k in range(2)]

    # --- h_sum = hc_even + hc_odd (strided), f32r; r-path emitted first so
    # the serial r -> rh -> Uc -> hcand chain starts as early as possible ---
    hs = [P["hs"].tile([128, Lp], f32r, name=f"hs{tag}_{m}", tag=f"hs{m}") for m in range(2)]
    for m in range(2):
        nc.vector.tensor_tensor(hs[m][:], hc3[m][:, :, 0::2], hc3[m][:, :, 1::2], ALU.add)

    # --- r = sigmoid(Ur @ h_sum + br) ---
    r = [P["r"].tile([128, Lp], f32, name=f"r{tag}_{m}", tag=f"r{m}") for m in range(2)]
    for m in range(2):
        lhs = [urT[k][:, m * 128:(m + 1) * 128] for k in range(2)]
        for p0 in range(0, Lp, PS_COLS):
            pn = min(PS_COLS, Lp - p0)
            ps = P["psrc"].tile([128, pn], f32, name=f"psr{tag}_{m}_{p0}", tag="psrc")
            for c0 in range(p0, p0 + pn, 512):
                n = min(512, p0 + pn - c0)
                mm_into(ps, c0 - p0, lhs, [hs[k][:, c0:c0 + n] for k in range(2)])
            nc.scalar.activation(r[m][:, p0:p0 + pn], ps[:], AF.Sigmoid, bias=br[m][:])

    # --- z = sigmoid(Uz @ hc + bz) over all children (fills PE while ACT r runs) ---
    z = [P["z"].tile([128, Lc], f32, name=f"z{tag}_{m}", tag=f"z{m}") for m in range(2)]
    for m in range(2):
        lhs = [uzT[k][:, m * 128:(m + 1) * 128] for k in range(2)]
        for p0 in range(0, Lc, PS_COLS):
            pn = min(PS_COLS, Lc - p0)
            ps = P["psz"].tile([128, pn], f32, name=f"psz{tag}_{m}_{p0}", tag="psz")
            for c0, rhs in child_chunks():
                if p0 <= c0 < p0 + pn:
                    mm_into(ps, c0 - p0, lhs, rhs)
            nc.scalar.activation(z[m][:, p0:p0 + pn], ps[:], AF.Sigmoid, bias=bz[m][:])

    # --- rh = r * h_sum (in place into hs, stays f32r) ---
    for m in range(2):
        nc.vector.tensor_tensor(hs[m][:], r[m][:], hs[m][:], ALU.mult)

    # --- h_cand = tanh(Uc @ rh + bc) ---
    hcand = [P["hc"].tile([128, Lp], f32, name=f"hcand{tag}_{m}", tag=f"hcand{m}") for m in range(2)]
    for m in range(2):
        lhs = [ucT[k][:, m * 128:(m + 1) * 128] for k in range(2)]
        for p0 in range(0, Lp, PS_COLS):
            pn = min(PS_COLS, Lp - p0)
            ps = P["psrc"].tile([128, pn], f32, name=f"psc{tag}_{m}_{p0}", tag="psrc")
            for c0 in range(p0, p0 + pn, 512):
                n = min(512, p0 + pn - c0)
                mm_into(ps, c0 - p0, lhs, [hs[k][:, c0:c0 + n] for k in range(2)])
            nc.scalar.activation(hcand[m][:, p0:p0 + pn], ps[:], AF.Tanh, bias=bc[m][:])

    for m in range(2):
        z3 = z[m][:].rearrange("p (t n) -> p t n", t=NT)
        # zs = z_even + z_odd  (before z is overwritten by zh); reuses the r slot
        zs = P["r"].tile([128, Lp], f32, name=f"zs{tag}_{m}", tag=f"r{m}")
        nc.vector.tensor_tensor(zs[:], z3[:, :, 0::2], z3[:, :, 1::2], ALU.add)
        # zh = z * hc, in place into z (DVE: gpsimd would contend for the
        # shared DVE/GpSimd SBUF port pair and slow both engines ~4x)
        nc.vector.tensor_tensor(z[m][:], z[m][:], hc3[m].bitcast(f32), ALU.mult)
        # zh_sum = zh_even + zh_odd; reuses the h_sum slot
        zhs = P["hs"].tile([128, Lp], f32, name=f"zhs{tag}_{m}", tag=f"hs{m}")
        nc.vector.tensor_tensor(zhs[:], z3[:, :, 0::2], z3[:, :, 1::2], ALU.add)
        # t = (zs - 1) * h_cand, in place into hcand
        nc.vector.scalar_tensor_tensor(hcand[m][:], zs[:], 1.0, hcand[m][:], ALU.subtract, ALU.mult)
        # h_new = zh_sum - t  -> out3 (f32r)
        nc.vector.tensor_tensor(out3[m], zhs[:], hcand[m][:], ALU.subtract)


def _build():
    nc = bacc.Bacc("TRN2", debug=False)

    xT_d = nc.dram_tensor("xT", [H, TPC * NLEAF], f32r, kind="ExternalInput")
    wT_d = nc.dram_tensor("wT", [H, H], f32r, kind="ExternalInput")
    uzT_d = nc.dram_tensor("uzT", [H, H], f32r, kind="ExternalInput")
    urT_d = nc.dram_tensor("urT", [H, H], f32r, kind="ExternalInput")
    ucT_d = nc.dram_tensor("ucT", [H, H], f32r, kind="ExternalInput")
    bw_d = nc.dram_tensor("bw", [H, 1], f32, kind="ExternalInput")
    bz_d = nc.dram_tensor("bz", [H, 1], f32, kind="ExternalInput")
    br_d = nc.dram_tensor("br", [H, 1], f32, kind="ExternalInput")
    bc_d = nc.dram_tensor("bc", [H, 1], f32, kind="ExternalInput")
    hout_d = nc.dram_tensor("h_out", [H, TPC, NN], f32, kind="ExternalOutput")

    with tile.TileContext(nc) as tc:
        from contextlib import ExitStack
        with ExitStack() as ctx:
            P = {}
            P["const"] = ctx.enter_context(tc.tile_pool(name="const", bufs=1))
            P["xg"] = ctx.enter_context(tc.tile_pool(name="xg", bufs=2))
            P["h10"] = ctx.enter_context(tc.tile_pool(name="h10", bufs=1))
            P["hl"] = ctx.enter_context(tc.tile_pool(name="hl", bufs=1))
            P["jbuf"] = ctx.enter_context(tc.tile_pool(name="jbuf", bufs=1))
            P["z"] = ctx.enter_context(tc.tile_pool(name="z", bufs=1))
            P["hs"] = ctx.enter_context(tc.tile_pool(name="hs", bufs=1))
            P["r"] = ctx.enter_context(tc.tile_pool(name="r", bufs=1))
            P["hc"] = ctx.enter_context(tc.tile_pool(name="hc", bufs=1))
            P["psz"] = ctx.enter_context(tc.tile_pool(name="psz", bufs=2, space="PSUM"))
            P["psrc"] = ctx.enter_context(tc.tile_pool(name="psrc", bufs=2, space="PSUM"))

            cp = P["const"]
            Wt = {}
            for nm, d in (("w", wT_d), ("uz", uzT_d), ("ur", urT_d), ("uc", ucT_d)):
                Wt[nm] = [cp.tile([128, H], f32r, name=f"{nm}T{k}") for k in range(2)]
                for k in range(2):
                    nc.sync.dma_start(Wt[nm][k][:], d.ap()[k * 128:(k + 1) * 128, :])
            bias = {}
            for nm, d in (("bw", bw_d), ("bz", bz_d), ("br", br_d), ("bc", bc_d)):
                bias[nm] = [cp.tile([128, 1], f32, name=f"{nm}{m}") for m in range(2)]
                for m in range(2):
                    nc.sync.dma_start(bias[nm][m][:], d.ap()[m * 128:(m + 1) * 128, :])

            # joint buffer: heap nodes 0..126 for all 16 trees, per half
            jbuf = [P["jbuf"].tile([128, TPC * JN], f32r, name=f"jbuf{m}") for m in range(2)]
            jv = [jbuf[m][:].rearrange("p (t n) -> p t n", t=TPC) for m in range(2)]

            for g in range(NG):
                gt = f"g{g}"
                # ---- leaf phase: h10 = tanh(W @ x + bw) ----
                xg = [P["xg"].tile([128, G * NLEAF], f32r, name=f"x{gt}_{k}", tag="xg")
                      for k in range(2)]
                for k in range(2):
                    nc.sync.dma_start(
                        xg[k][:],
                        xT_d.ap()[k * 128:(k + 1) * 128,
                                  g * G * NLEAF:(g + 1) * G * NLEAF])
                h10 = [P["h10"].tile([128, G * NLEAF], f32r, name=f"h10{gt}_{m}", tag=f"h10{m}")
                       for m in range(2)]
                for m in range(2):
                    lhs = [Wt["w"][k][:, m * 128:(m + 1) * 128] for k in range(2)]
                    for p0 in range(0, G * NLEAF, PS_COLS):
                        pn = min(PS_COLS, G * NLEAF - p0)
                        ps = P["psz"].tile([128, pn], f32, name=f"psx{gt}_{m}_{p0}", tag="psz")
                        for c0 in range(p0, p0 + pn, 512):
                            n = min(512, p0 + pn - c0)
                            nc.tensor.matmul(ps[:, c0 - p0:c0 - p0 + n], lhs[0],
                                             xg[0][:, c0:c0 + n], start=True, stop=False)
                            nc.tensor.matmul(ps[:, c0 - p0:c0 - p0 + n], lhs[1],
                                             xg[1][:, c0:c0 + n], start=False, stop=True)
                        nc.scalar.activation(h10[m][:, p0:p0 + pn], ps[:], AF.Tanh,
                                             bias=bias["bw"][m][:])
                    nc.sync.dma_start(
                        hout_d.ap()[m * 128:(m + 1) * 128, g * G:(g + 1) * G,
                                    LEAF0:LEAF0 + NLEAF],
                        h10[m][:].rearrange("p (t n) -> p t n", t=G).bitcast(f32))

                # ---- levels 9..6 for this group ----
                hchild = h10
                for lv in range(DEPTH - 2, 5, -1):
                    Lct = 2 ** (lv + 1)
                    Lpt = 2 ** lv
                    hc3 = [hchild[m][:].rearrange("p (t n) -> p t n", t=G) for m in range(2)]
                    hc_flat = [hchild[m][:] for m in range(2)]
                    if lv == 6:
                        out3 = [jv[m][:, g * G:(g + 1) * G, Lpt - 1:2 * Lpt - 1]
                                for m in range(2)]
                    else:
                        hnew = [P["hl"].tile([128, G * Lpt], f32r,
                                             name=f"h{lv}{gt}_{m}", tag=f"h{lv}_{m}")
                                for m in range(2)]
                        out3 = [hnew[m][:].rearrange("p (t n) -> p t n", t=G)
                                for m in range(2)]
                    _emit_level(nc, P, f"{gt}l{lv}", G, Lct, hc3, hc_flat, out3, Wt, bias)
                    if lv > 6:
                        for m in range(2):
                            nc.sync.dma_start(
                                hout_d.ap()[m * 128:(m + 1) * 128, g * G:(g + 1) * G,
                                            Lpt - 1:2 * Lpt - 1],
                                hnew[m][:].rearrange("p (t n) -> p t n", t=G).bitcast(f32))
                        hchild = hnew

            # ---- joint levels 5..0 over jbuf ----
            for lv in range(5, -1, -1):
                Lct = 2 ** (lv + 1)
                Lpt = 2 ** lv
                hc3 = [jv[m][:, :, Lct - 1:2 * Lct - 1] for m in range(2)]
                out3 = [jv[m][:, :, Lpt - 1:2 * Lpt - 1] for m in range(2)]
                _emit_level(nc, P, f"j{lv}", TPC, Lct, hc3, None, out3, Wt, bias)

            for m in range(2):
                nc.sync.dma_start(
                    hout_d.ap()[m * 128:(m + 1) * 128, :, 0:JN],
                    jv[m].bitcast(f32))

    nc.compile()
    return nc


_NC = None


def _get_nc():
    global _NC
    if _NC is None:
        _NC = _build()
    return _NC


def make_in_maps(inputs):
    x = np.asarray(inputs["x"], np.float32)
    W = np.asarray(inputs["W"], np.float32)
    bW = np.asarray(inputs["bW"], np.float32).reshape(H, 1)
    Ur = np.asarray(inputs["Ur"], np.float32)
    br = np.asarray(inputs["br"], np.float32).reshape(H, 1)
    Uc = np.asarray(inputs["Uc"], np.float32)
    bc = np.asarray(inputs["bc"], np.float32).reshape(H, 1)
    Uz = np.asarray(inputs["Uz"], np.float32)
    bz = np.asarray(inputs["bz"], np.float32).reshape(H, 1)
    shared = {
        "wT": np.ascontiguousarray(W.T), "uzT": np.ascontiguousarray(Uz.T),
        "urT": np.ascontiguousarray(Ur.T), "ucT": np.ascontiguousarray(Uc.T),
        "bw": bW, "bz": bz, "br": br, "bc": bc,
    }
    in_maps = []
    for c in range(NCORES):
        xs = x[c * TPC:(c + 1) * TPC, LEAF0:, :]          # [16, 1024, 256]
        xTc = np.ascontiguousarray(xs.transpose(2, 0, 1)).reshape(H, TPC * NLEAF)
        in_maps.append({"xT": xTc, **shared})
    return in_maps


def assemble_out(core_outs):
    out = np.empty((T, NN, H), np.float32)
    for c in range(NCORES):
        # [256, 16, 2047] -> [16, 2047, 256]
        out[c * TPC:(c + 1) * TPC] = core_outs[c].transpose(1, 2, 0)
    return out


def kernel(**inputs):
    nc = _get_nc()
    in_maps = make_in_maps(inputs)
    res = run_bass_kernel_spmd(nc, in_maps, list(range(NCORES)))
    return assemble_out([r["h_out"] for r in res.results])
